# revision 1
# baseline (speedup 1.0000x reference)
"""Trainium2 Bass kernel for nn_Net2_54494545051831 (LocallyConnected2d(7x7)
-> bias -> ReLU -> Linear(28392 -> 10)), on 8 NeuronCores.

Distribution: by output location. Each core owns 3 full output rows
(h = 3c .. 3c+2) plus a 6-7 wide piece of rows 24/25 -> 84/85 locations.
Weights / bias / lw are sharded by location (nothing replicated); each core
computes a partial [10, B] of the final linear layer, summed on host.

Per-core compute ("band" layout): for each owned output row, x is reordered
host-side so the contraction rows of location (h, w) sit at band partitions
[32w, 32w+224): band row 32*w' + j = x[:, j//7, h + j%7, w'] for j < 21,
1.0 at j == 21 (bias folds into the weights), 0 above. A location is then
2-3 matmuls with 32-aligned partition windows (zero weights on pad rows).
Two locations run concurrently in the two column halves of the PE array
(M padded 42 -> 64 so the whole [128, 512] psum block relus in one op and
garbage rows are exact zeros). All matmuls bf16 with fp32 accumulation.
"""
import numpy as np
import ml_dtypes

import concourse.mybir as mybir
import concourse.tile as tile
from concourse import bacc
from concourse.bass_utils import run_bass_kernel_spmd

BF16 = mybir.dt.bfloat16
F32 = mybir.dt.float32
RELU = mybir.ActivationFunctionType.Relu

B = 1024
IC, OC, NCLS = 3, 42, 10
KH = KW = 7
OH = OW = 26
NCORES = 8
N_ROWS = 4           # canonical band rows per core (3 full + 1 piece)
STRIDE = 22          # band rows per w'-block: 21 data + 1 ones(bias) row
WINLEN = 6 * STRIDE + 21   # partition span of one location window (153)
TPR = 6              # band tiles per canonical row (704 rows -> 6 tiles)
N_SLOT = 86          # 3*26 + 8 canonical slots (last is always pad)
N_PAIR = 43          # 43 full pairs (86 slots)
NB = 2               # two N-chunks of 512
NCHUNK = 512
MPAD = 64            # output channels padded 42 -> 64

MAX_CHUNKS = 3
WT_COLS = N_SLOT * MAX_CHUNKS * MPAD          # weight sbuf free dim
N_BTILE = N_ROWS * TPR                        # band tiles per core


def _slot_tiles(w_c):
    ta = (STRIDE * w_c) // 128
    tb = (STRIDE * w_c + WINLEN - 1) // 128
    return list(range(ta, tb + 1))

_cache = {}


def _slot_geo(i):
    """Canonical slot -> (band_row, w_c). Rows 0-2 full, row 3 has 8 slots."""
    if i < 78:
        return i // 26, i % 26
    return 3, i - 78


def _build_program():
    if "nc" in _cache:
        return _cache["nc"], _cache["names"]

    nc = bacc.Bacc("TRN2", target_bir_lowering=False, debug=False,
                   num_devices=NCORES)
    band_d = nc.dram_tensor("band", [N_BTILE, 128, B], BF16,
                            kind="ExternalInput").ap()
    wt_d = nc.dram_tensor("wt", [128, WT_COLS], BF16,
                          kind="ExternalInput").ap()
    lwp_d = nc.dram_tensor("lwp", [128, N_PAIR * NCLS], BF16,
                           kind="ExternalInput").ap()
    part_d = nc.dram_tensor("part", [NCLS, B], F32,
                            kind="ExternalOutput").ap()

    with tile.TileContext(nc) as tc:
        with (
            tc.tile_pool(name="sb", bufs=1) as sb,
            tc.tile_pool(name="stk_pool", bufs=10) as stk_pool,
            tc.tile_pool(name="pp_pool", bufs=5, space="PSUM") as pp_pool,
            tc.tile_pool(name="lin_pool", bufs=1, space="PSUM") as lin_pool,
        ):
            band_s = sb.tile([128, N_BTILE * B], BF16)
            wt_s = sb.tile([128, WT_COLS], BF16)
            lwp_s = sb.tile([128, N_PAIR * NCLS], BF16)
            out_s = sb.tile([NCLS, NB * NCHUNK], F32)

            # DMA order: row-0 band tiles + first weight chunk first so
            # the PE can start immediately; rest follows interleaved.
            wchunk = WT_COLS // 8
            def dma_band(t):
                nc.sync.dma_start(
                    out=band_s[:, t * B:(t + 1) * B], in_=band_d[t]
                )
            def dma_wt(d):
                nc.sync.dma_start(
                    out=wt_s[:, d * wchunk:(d + 1) * wchunk],
                    in_=wt_d[:, d * wchunk:(d + 1) * wchunk],
                )
            for t in range(TPR):
                dma_band(t)
            dma_wt(0)
            nc.sync.dma_start(out=lwp_s, in_=lwp_d)
            for r in range(1, N_ROWS):
                for t in range(TPR):
                    dma_band(r * TPR + t)
                for d in range(1 + (r - 1) * 3, min(8, 1 + r * 3)):
                    dma_wt(d)
            for d in range(7, 8):
                dma_wt(d)

            lin_ps = [lin_pool.tile([NCLS, NCHUNK], F32, tag=f"lin{nb}",
                                    name=f"lin_ps{nb}")
                      for nb in range(NB)]

            NBLK = (N_PAIR + 1) // 2
            LIN_LAG = 8          # defer linear MMs by this many pairs

            for nb in range(NB):
                pend = []

                def _flush_lin(limit, nb=nb, pend=None):
                    pass

                def emit_lin(pi, stk, qi, nb=nb):
                    nc.tensor.matmul(
                        lin_ps[nb],
                        lwp_s[:, pi * NCLS:(pi + 1) * NCLS],
                        stk[:, qi * NCHUNK:(qi + 1) * NCHUNK],
                        start=(pi == 0), stop=(pi == N_PAIR - 1),
                        tile_position=(0, 0), skip_group_check=True,
                    )

                for blk in range(N_PAIR):
                    bpairs = [blk]
                    pp = pp_pool.tile([128, 1 * NCHUNK], F32, tag="pp")
                    for qi, pi in enumerate(bpairs):
                        for side in range(2):
                            sl = 2 * pi + side
                            row, w_c = _slot_geo(sl)
                            tiles = _slot_tiles(w_c)
                            nchunks = len(tiles)
                            for ci, tc in enumerate(tiles):
                                btile = row * TPR + tc
                                rhs = band_s[
                                    :,
                                    btile * B + nb * NCHUNK:
                                    btile * B + nb * NCHUNK + NCHUNK,
                                ]
                                lhsT = wt_s[
                                    :,
                                    (sl * MAX_CHUNKS + ci) * MPAD:
                                    (sl * MAX_CHUNKS + ci + 1) * MPAD,
                                ]
                                nc.tensor.matmul(
                                    pp[MPAD * side:MPAD * (side + 1),
                                       qi * NCHUNK:(qi + 1) * NCHUNK],
                                    lhsT, rhs,
                                    start=(ci == 0),
                                    stop=(ci == nchunks - 1),
                                    tile_position=(0, MPAD * side),
                                )
                    stk = stk_pool.tile([128, 2 * NCHUNK], BF16, tag="stk")
                    width = len(bpairs) * NCHUNK
                    if blk % 2 == 0:
                        nc.vector.tensor_scalar_max(
                            stk[:, 0:width], pp[:, 0:width], 0.0)
                    else:
                        nc.scalar.activation(
                            stk[:, 0:width], pp[:, 0:width], RELU)
                    for qi, pi in enumerate(bpairs):
                        pend.append((pi, stk, qi))
                    while len(pend) > LIN_LAG:
                        emit_lin(*pend.pop(0))
                while pend:
                    emit_lin(*pend.pop(0))
                nc.vector.tensor_copy(
                    out_s[:, nb * NCHUNK:(nb + 1) * NCHUNK], lin_ps[nb]
                )
            nc.sync.dma_start(out=part_d, in_=out_s)

    nc.compile()
    names = dict(band="band", wt="wt", lwp="lwp", part="part")
    _cache["nc"] = nc
    _cache["names"] = names
    return nc, names


def _core_slots(c):
    """Actual (h, w) per canonical slot for core c; None = pad."""
    slots = []
    for i in range(78):
        slots.append((3 * c + i // 26, i % 26))
    p0 = (52 * c) // 8
    p1 = (52 * (c + 1)) // 8
    ph, pw0 = 24 + p0 // 26, p0 % 26
    plen = p1 - p0
    for j in range(N_SLOT - 78):
        slots.append((ph, pw0 + j) if j < plen else None)
    return slots, (ph, pw0, plen)


def _prep_core(c, x, W, b, lw4):
    """Build band / wt / lwp arrays for core c."""
    slots, (ph, pw0, plen) = _core_slots(c)

    # bands ------------------------------------------------------------
    # canonical row r: actual output row h_r with block shift s_r
    hs = [(3 * c, 0), (3 * c + 1, 0), (3 * c + 2, 0), (ph, pw0)]
    band = np.zeros((N_BTILE, 128, B), dtype=ml_dtypes.bfloat16)
    cj = np.arange(21) // 7          # channel per j
    kij = np.arange(21) % 7          # kernel-row per j
    for r, (h, shift) in enumerate(hs):
        nblocks = min(32, 32 - shift)
        # blk[j, bw, :] = x[:, cj, h+kij, shift+bw]
        wslice = np.arange(nblocks) + shift
        blk = x[:, cj[:, None], (h + kij)[:, None], wslice[None, :]]
        blk = blk.transpose(1, 2, 0)          # [21, nblocks, B]
        brow = np.zeros((TPR * 128, B), dtype=ml_dtypes.bfloat16)
        for bw in range(nblocks):
            brow[STRIDE * bw:STRIDE * bw + 21] = blk[:, bw]
            brow[STRIDE * bw + 21] = 1.0
        band[r * TPR:(r + 1) * TPR] = brow.reshape(TPR, 128, B)
    # wt ----------------------------------------------------------------
    wt = np.zeros((128, WT_COLS), dtype=ml_dtypes.bfloat16)
    for sl, hw in enumerate(slots):
        if hw is None:
            continue
        h, w = hw
        _, w_c = _slot_geo(sl)
        Wl = W[:, :, h, w, :]                 # [42, 3, 49]
        bl = b[:, h, w]                       # [42]
        for ci, tc in enumerate(_slot_tiles(w_c)):
            col = (sl * MAX_CHUNKS + ci) * MPAD
            rel = 128 * tc + np.arange(128) - STRIDE * w_c
            kj = rel // STRIDE
            jj = rel % STRIDE
            valid = (rel >= 0) & (rel < WINLEN) & (jj < 21)
            vals = np.zeros((128, OC), dtype=np.float32)
            vj, vk = jj[valid], kj[valid]
            vals[valid] = Wl[:, vj // 7, (vj % 7) * 7 + vk].T
            bias_row = (rel >= 0) & (rel < WINLEN) & (jj == 21) & (kj == 0)
            if bias_row.any():
                vals[bias_row] = bl
            wt[:, col:col + OC] = vals.astype(ml_dtypes.bfloat16)
    # lwp ---------------------------------------------------------------
    lwp = np.zeros((128, N_PAIR * NCLS), dtype=ml_dtypes.bfloat16)
    for pi in range(N_PAIR):
        for side in range(2):
            sl = 2 * pi + side
            if slots[sl] is None:
                continue
            h, w = slots[sl]
            lwp[MPAD * side:MPAD * side + OC, pi * NCLS:(pi + 1) * NCLS] = (
                lw4[:, :, h, w].T.astype(ml_dtypes.bfloat16)
            )
    return {"band": band, "wt": wt, "lwp": lwp}


def _run(x, W, b, lw, lb, trace=False):
    nc, _ = _build_program()
    x = np.ascontiguousarray(np.asarray(x, dtype=np.float32))
    W = np.asarray(W, dtype=np.float32)
    b = np.asarray(b, dtype=np.float32)
    lw = np.asarray(lw, dtype=np.float32)
    lb = np.asarray(lb, dtype=np.float32)
    lw4 = lw.reshape(NCLS, OC, OH, OW)
    in_maps = [_prep_core(c, x, W, b, lw4) for c in range(NCORES)]
    res = run_bass_kernel_spmd(
        nc, in_maps, list(range(NCORES)), trace=trace,
    )
    part = np.zeros((NCLS, B), dtype=np.float32)
    for c in range(NCORES):
        part += res.results[c]["part"]
    out = part.T + lb[None, :]
    return out.astype(np.float32), res


def kernel(**inputs):
    out, _ = _run(inputs["x"], inputs["W"], inputs["b"], inputs["lw"],
                  inputs["lb"])
    return out



# revision 4
# speedup vs baseline: 1.1494x; 1.1494x over previous
"""Trainium2 Bass kernel for nn_Net2_54494545051831 (LocallyConnected2d(7x7)
-> bias -> ReLU -> Linear(28392 -> 10)), on 8 NeuronCores.

Distribution: by output location. Each core owns 3 full output rows
(h = 3c .. 3c+2) plus a 6-7 wide piece of rows 24/25 -> 84/85 locations.
Weights / bias / lw are sharded by location (nothing replicated); each core
computes a partial [10, B] of the final linear layer, summed on host.

Per-core compute ("band" layout): for each owned output row, x is reordered
host-side so the contraction rows of location (h, w) sit at band partitions
[22w, 22w+153): band row 22*w' + j = x[:, j//7, h + j%7, w'] for j < 21,
1.0 at j == 21 (bias folds into the weights), 0 above.

Locations are processed in GROUPS of 3 (42*3 = 126 output channels + 2 zero
columns = full 128-wide stationary operand). A group's 3 windows span
[22w0, 22w0+197) -> 2-3 aligned 128-row band tiles; one full-width matmul
per (group, tile) with zero weights on rows outside each location's window.
ReLU round-robins across Vector/Scalar/GpSimd; the linear layer contracts
each group's relu'd [128, 512] tile with a [128, 10] per-group lw slice,
accumulating in PSUM. bf16 matmuls with fp32 accumulation.
"""
import numpy as np
import ml_dtypes

import concourse.mybir as mybir
import concourse.tile as tile
from concourse import bacc
from concourse.bass_utils import run_bass_kernel_spmd

BF16 = mybir.dt.bfloat16
F32 = mybir.dt.float32
RELU = mybir.ActivationFunctionType.Relu

B = 1024
IC, OC, NCLS = 3, 42, 10
KH = KW = 7
OH = OW = 26
NCORES = 8
N_ROWS = 4           # canonical band rows per core (3 full + 1 piece)
STRIDE = 22          # band rows per w'-block: 21 data + 1 ones(bias) row
WINLEN = 6 * STRIDE + 21   # partition span of one location window (153)
TPR = 6              # band tiles per canonical row (704 rows -> 6 tiles)
NB = 2               # two N-chunks of 512
NCHUNK = 512
N_WARM = 16          # PE warm-up matmuls during the DMA prologue

# Groups of consecutive locations within a canonical row: (w0, len)
GROUPS_FULL = [(0, 3), (3, 3), (6, 3), (9, 3), (12, 3), (15, 3), (18, 3),
               (21, 3), (24, 2)]
GROUPS_ROW3 = [(0, 3), (3, 3), (6, 2)]


def _group_tiles(w0, L):
    ta = (STRIDE * w0) // 128
    tb = (STRIDE * (w0 + L - 1) + WINLEN - 1) // 128
    return list(range(ta, tb + 1))


def _groups():
    """[(row, w0, L, [tiles], chunk0)] — chunk0 = first wt chunk index."""
    out = []
    ck = 0
    for r in range(N_ROWS):
        for w0, L in (GROUPS_FULL if r < 3 else GROUPS_ROW3):
            ts = _group_tiles(w0, L)
            out.append((r, w0, L, ts, ck))
            ck += len(ts)
    return out, ck

GROUPS, N_CHUNK_TOT = _groups()
NG = len(GROUPS)
# band tiles actually used per canonical row (row 3 only needs tiles 0-2)
ROW_TILES = [TPR, TPR, TPR, max(t for (r, _, _, ts, _) in GROUPS if r == 3
                                for t in ts) + 1]

_cache = {}


def _build_program():
    if "nc" in _cache:
        return _cache["nc"]

    nc = bacc.Bacc("TRN2", target_bir_lowering=False, debug=False,
                   num_devices=NCORES)
    band_d = nc.dram_tensor("band", [N_ROWS * TPR, 128, B], BF16,
                            kind="ExternalInput").ap()
    wt_d = nc.dram_tensor("wt", [128, N_CHUNK_TOT * 128], BF16,
                          kind="ExternalInput").ap()
    lwp_d = nc.dram_tensor("lwp", [128, NG * NCLS], BF16,
                           kind="ExternalInput").ap()
    part_d = nc.dram_tensor("part", [NCLS, B], F32,
                            kind="ExternalOutput").ap()

    with tile.TileContext(nc) as tc:
        with (
            tc.tile_pool(name="sb", bufs=1) as sb,
            tc.tile_pool(name="stk_pool", bufs=8) as stk_pool,
            tc.tile_pool(name="pp_pool", bufs=5, space="PSUM") as pp_pool,
            tc.tile_pool(name="lin_pool", bufs=1, space="PSUM") as lin_pool,
            tc.tile_pool(name="warm_pool", bufs=1, space="PSUM") as warm_pool,
        ):
            band_s = sb.tile([128, N_ROWS * TPR * B], BF16)
            wt_s = sb.tile([128, N_CHUNK_TOT * 128], BF16)
            lwp_s = sb.tile([128, NG * NCLS], BF16)
            out_s = sb.tile([NCLS, NB * NCHUNK], F32)
            zz = sb.tile([128, NCHUNK], BF16)

            # PE warm-up: zero tile matmuls with no DMA deps — scheduled
            # during the DMA prologue so HAM reaches K=8/8 before real work.
            nc.gpsimd.memset(zz, 0.0)
            warm_ps = warm_pool.tile([128, NCHUNK], F32, name="warm_ps")
            for _ in range(N_WARM):
                nc.tensor.matmul(warm_ps, zz[:, 0:128], zz,
                                 start=True, stop=True)

            # DMA issuance in consumption order, triggers spread across
            # engines so Sync isn't the serial bottleneck.
            dma_rr = [nc.sync, nc.scalar, nc.gpsimd]
            dma_i = [0]

            def dma(out, in_):
                eng = dma_rr[dma_i[0] % len(dma_rr)]
                dma_i[0] += 1
                eng.dma_start(out=out, in_=in_)

            def dma_band(r, t):
                bt = r * TPR + t
                dma(band_s[:, bt * B:(bt + 1) * B], band_d[bt])

            def dma_wt(c0, c1):
                dma(wt_s[:, c0 * 128:c1 * 128], wt_d[:, c0 * 128:c1 * 128])

            row_groups = [[g for g in GROUPS if g[0] == r]
                          for r in range(N_ROWS)]
            for r in range(N_ROWS):
                gs = row_groups[r]
                dma_band(r, 0)
                dma_band(r, 1)
                dma_wt(gs[0][4], gs[0][4] + len(gs[0][3]))
                if r == 0:
                    dma(lwp_s, lwp_d)
                nxt = 2
                for g in gs[1:]:
                    need = g[3][-1] + 1
                    while nxt < min(need, ROW_TILES[r]):
                        dma_band(r, nxt)
                        nxt += 1
                    dma_wt(g[4], g[4] + len(g[3]))
                while nxt < ROW_TILES[r]:
                    dma_band(r, nxt)
                    nxt += 1

            lin_ps = [lin_pool.tile([NCLS, NCHUNK], F32, name=f"lin_ps{nb}")
                      for nb in range(NB)]

            LAG = 3
            pend = []
            lin_cnt = [0, 0]

            def emit_lin(gi, nb, stk):
                lin_cnt[nb] += 1
                nc.tensor.matmul(
                    lin_ps[nb],
                    lwp_s[:, gi * NCLS:(gi + 1) * NCLS],
                    stk,
                    start=(gi == 0), stop=(gi == NG - 1),
                    skip_group_check=True,
                )

            relu_i = [0]

            def emit_relu(stk, pp):
                k = relu_i[0] % 2
                relu_i[0] += 1
                if k == 0:
                    nc.vector.tensor_scalar_max(stk, pp, 0.0)
                else:
                    nc.scalar.activation(stk, pp, RELU)

            for r in range(N_ROWS):
                for nb in range(NB):
                    for (gr, w0, L, ts, ck) in row_groups[r]:
                        gi = GROUPS.index((gr, w0, L, ts, ck))
                        pp = pp_pool.tile([128, NCHUNK], F32, tag="pp")
                        for ci, t in enumerate(ts):
                            bt = r * TPR + t
                            nc.tensor.matmul(
                                pp,
                                wt_s[:, (ck + ci) * 128:(ck + ci + 1) * 128],
                                band_s[:, bt * B + nb * NCHUNK:
                                       bt * B + nb * NCHUNK + NCHUNK],
                                start=(ci == 0), stop=(ci == len(ts) - 1),
                            )
                        stk = stk_pool.tile([128, NCHUNK], BF16, tag="stk")
                        emit_relu(stk, pp)
                        pend.append((gi, nb, stk))
                        while len(pend) > LAG:
                            emit_lin(*pend.pop(0))
            while pend:
                emit_lin(*pend.pop(0))
            for nb in range(NB):
                nc.vector.tensor_copy(
                    out_s[:, nb * NCHUNK:(nb + 1) * NCHUNK], lin_ps[nb])
            nc.sync.dma_start(out=part_d, in_=out_s)

    nc.compile()
    _cache["nc"] = nc
    return nc


def _core_slots(c):
    """Actual (h, w) per canonical slot for core c; None = pad."""
    slots = []
    for i in range(78):
        slots.append((3 * c + i // 26, i % 26))
    p0 = (52 * c) // 8
    p1 = (52 * (c + 1)) // 8
    ph, pw0 = 24 + p0 // 26, p0 % 26
    plen = p1 - p0
    for j in range(8):
        slots.append((ph, pw0 + j) if j < plen else None)
    return slots, (ph, pw0, plen)


def _prep_core(c, x, W, b, lw4):
    """Build band / wt / lwp arrays for core c."""
    slots, (ph, pw0, plen) = _core_slots(c)

    # bands ------------------------------------------------------------
    hs = [(3 * c, 0), (3 * c + 1, 0), (3 * c + 2, 0), (ph, pw0)]
    band = np.zeros((N_ROWS * TPR, 128, B), dtype=ml_dtypes.bfloat16)
    cj = np.arange(21) // 7          # channel per j
    kij = np.arange(21) % 7          # kernel-row per j
    for r, (h, shift) in enumerate(hs):
        nblocks = min(32, 32 - shift)
        wslice = np.arange(nblocks) + shift
        blk = x[:, cj[:, None], (h + kij)[:, None], wslice[None, :]]
        blk = blk.transpose(1, 2, 0)          # [21, nblocks, B]
        brow = np.zeros((TPR * 128, B), dtype=ml_dtypes.bfloat16)
        for bw in range(nblocks):
            brow[STRIDE * bw:STRIDE * bw + 21] = blk[:, bw]
            brow[STRIDE * bw + 21] = 1.0
        band[r * TPR:(r + 1) * TPR] = brow.reshape(TPR, 128, B)
    # wt ----------------------------------------------------------------
    wt = np.zeros((128, N_CHUNK_TOT * 128), dtype=ml_dtypes.bfloat16)
    for (r, w0, L, ts, ck) in GROUPS:
        for s in range(L):
            w_c = w0 + s
            sl = r * 26 + w_c if r < 3 else 78 + w_c
            hw = slots[sl]
            if hw is None:
                continue
            h, w = hw
            Wl = W[:, :, h, w, :]                 # [42, 3, 49]
            bl = b[:, h, w]                       # [42]
            for ci, t in enumerate(ts):
                col = (ck + ci) * 128 + 42 * s
                rel = 128 * t + np.arange(128) - STRIDE * w_c
                kj = rel // STRIDE
                jj = rel % STRIDE
                valid = (rel >= 0) & (rel < WINLEN) & (jj < 21)
                vals = np.zeros((128, OC), dtype=np.float32)
                vj, vk = jj[valid], kj[valid]
                vals[valid] = Wl[:, vj // 7, (vj % 7) * 7 + vk].T
                bias_row = (rel >= 0) & (rel < WINLEN) & (jj == 21) & (kj == 0)
                if bias_row.any():
                    vals[bias_row] = bl
                wt[:, col:col + OC] = vals.astype(ml_dtypes.bfloat16)
    # lwp ---------------------------------------------------------------
    lwp = np.zeros((128, NG * NCLS), dtype=ml_dtypes.bfloat16)
    for gi, (r, w0, L, ts, ck) in enumerate(GROUPS):
        for s in range(L):
            w_c = w0 + s
            sl = r * 26 + w_c if r < 3 else 78 + w_c
            if slots[sl] is None:
                continue
            h, w = slots[sl]
            lwp[42 * s:42 * s + OC, gi * NCLS:(gi + 1) * NCLS] = (
                lw4[:, :, h, w].T.astype(ml_dtypes.bfloat16)
            )
    return {"band": band, "wt": wt, "lwp": lwp}


def _run(x, W, b, lw, lb, trace=False):
    nc = _build_program()
    x = np.ascontiguousarray(np.asarray(x, dtype=np.float32))
    W = np.asarray(W, dtype=np.float32)
    b = np.asarray(b, dtype=np.float32)
    lw = np.asarray(lw, dtype=np.float32)
    lb = np.asarray(lb, dtype=np.float32)
    lw4 = lw.reshape(NCLS, OC, OH, OW)
    in_maps = [_prep_core(c, x, W, b, lw4) for c in range(NCORES)]
    res = run_bass_kernel_spmd(
        nc, in_maps, list(range(NCORES)), trace=trace,
    )
    part = np.zeros((NCLS, B), dtype=np.float32)
    for c in range(NCORES):
        part += res.results[c]["part"]
    out = part.T + lb[None, :]
    return out.astype(np.float32), res


def kernel(**inputs):
    out, _ = _run(inputs["x"], inputs["W"], inputs["b"], inputs["lw"],
                  inputs["lb"])
    return out


# revision 10
# speedup vs baseline: 1.4180x; 1.2336x over previous
"""Trainium2 Bass kernel for nn_Net2_54494545051831 (LocallyConnected2d(7x7)
-> bias -> ReLU -> Linear(28392 -> 10)), on 8 NeuronCores.

Distribution: by output location. Each core owns 3 full output rows
(h = 3c .. 3c+2) plus a 6-7 wide piece of rows 24/25 -> 84/85 locations.
Weights / bias / lw are sharded by location (nothing replicated); each core
computes a partial [10, B] of the final linear layer, summed on host.

Per-core compute ("band" layout): for each owned output row, x is reordered
host-side so the contraction rows of location (h, w) sit at band partitions
[22w, 22w+153): band row 22*w' + j = x[:, j//7, h + j%7, w'] for j < 21,
1.0 at j == 21 (bias folds into the weights), 0 above.

Locations are processed in GROUPS of 3 (42*3 = 126 output channels + 2 zero
columns = full 128-wide stationary operand). A group's 3 windows span
[22w0, 22w0+197) -> 2-3 aligned 128-row band tiles; one full-width matmul
per (group, tile) with zero weights on rows outside each location's window.
ReLU round-robins across Vector/Scalar/GpSimd; the linear layer contracts
each group's relu'd [128, 512] tile with a [128, 10] per-group lw slice,
accumulating in PSUM. bf16 matmuls with fp32 accumulation.
"""
import numpy as np
import ml_dtypes

import concourse.mybir as mybir
import concourse.tile as tile
from concourse import bacc
from concourse.bass_utils import run_bass_kernel_spmd

BF16 = mybir.dt.bfloat16
F32 = mybir.dt.float32
RELU = mybir.ActivationFunctionType.Relu

B = 1024
IC, OC, NCLS = 3, 42, 10
KH = KW = 7
OH = OW = 26
NCORES = 8
N_ROWS = 4           # canonical band rows per core (3 full + 1 piece)
STRIDE = 22          # band rows per w'-block: 21 data + 1 ones(bias) row
WINLEN = 6 * STRIDE + 21   # partition span of one location window (153)
TPR = 6              # band tiles per canonical row (704 rows -> 6 tiles)
NB = 2               # two N-chunks of 512
NCHUNK = 512
N_WARM = 16          # PE warm-up matmuls during the DMA prologue

# Groups of consecutive locations within a canonical row: (w0, len)
GROUPS_FULL = [(0, 3), (3, 3), (6, 3), (9, 3), (12, 3), (15, 3), (18, 3),
               (21, 3), (24, 2)]
GROUPS_ROW3 = [(0, 3), (3, 3), (6, 2)]


def _group_tiles(w0, L):
    ta = (STRIDE * w0) // 128
    tb = (STRIDE * (w0 + L - 1) + WINLEN - 1) // 128
    return list(range(ta, tb + 1))


def _groups():
    """[(row, w0, L, [tiles], chunk0)] — chunk0 = first wt chunk index."""
    out = []
    ck = 0
    for r in range(N_ROWS):
        for w0, L in (GROUPS_FULL if r < 3 else GROUPS_ROW3):
            ts = _group_tiles(w0, L)
            out.append((r, w0, L, ts, ck))
            ck += len(ts)
    return out, ck

GROUPS, N_CHUNK_TOT = _groups()
NG = len(GROUPS)
# band tiles actually used per canonical row (row 3 only needs tiles 0-2)
ROW_TILES = [TPR, TPR, TPR, max(t for (r, _, _, ts, _) in GROUPS if r == 3
                                for t in ts) + 1]

_cache = {}


def _build_program():
    if "nc" in _cache:
        return _cache["nc"]

    nc = bacc.Bacc("TRN2", target_bir_lowering=False, debug=False,
                   num_devices=NCORES)
    band_d = nc.dram_tensor("band", [N_ROWS * TPR, 128, B], BF16,
                            kind="ExternalInput").ap()
    wt_d = nc.dram_tensor("wt", [128, N_CHUNK_TOT * 128], BF16,
                          kind="ExternalInput").ap()
    lwp_d = nc.dram_tensor("lwp", [128, NG * NCLS], BF16,
                           kind="ExternalInput").ap()
    # 4 col-tile partial slices per nb chunk, summed on host
    part_d = nc.dram_tensor("part", [NB, 106, NCHUNK], F32,
                            kind="ExternalOutput").ap()

    with tile.TileContext(nc) as tc:
        with (
            tc.tile_pool(name="sb", bufs=1) as sb,
            tc.tile_pool(name="stk_pool", bufs=8) as stk_pool,
            tc.tile_pool(name="pp_pool", bufs=5, space="PSUM") as pp_pool,
            tc.tile_pool(name="lin_pool", bufs=1, space="PSUM") as lin_pool,
            tc.tile_pool(name="warm_pool", bufs=1, space="PSUM") as warm_pool,
        ):
            band_s = sb.tile([128, N_ROWS * TPR * B], BF16)
            wt_s = sb.tile([128, N_CHUNK_TOT * 128], BF16)
            lwp_s = sb.tile([128, NG * NCLS], BF16)
            zz = sb.tile([128, NCHUNK], BF16)

            # PE warm-up: zero tile matmuls with no DMA deps — scheduled
            # during the DMA prologue so HAM reaches K=8/8 before real work.
            nc.gpsimd.memset(zz, 0.0)
            warm_ps = warm_pool.tile([128, NCHUNK], F32, name="warm_ps")
            for _ in range(N_WARM):
                nc.tensor.matmul(warm_ps, zz[:, 0:128], zz,
                                 start=True, stop=True)

            # DMA: few large transfers in consumption order. Per canonical
            # row: band halves on sync/scalar, weights on gpsimd — each
            # engine's ring serializes its own transfers, so rows arrive
            # roughly in order and row r+1 never starves row r.
            row_groups = [[g for g in GROUPS if g[0] == r]
                          for r in range(N_ROWS)]

            def dma_band_range(eng, r, t0, t1):
                if t0 >= t1:
                    return
                b0 = r * TPR + t0
                eng.dma_start(out=band_s[:, b0 * B:(r * TPR + t1) * B],
                              in_=band_d[b0:r * TPR + t1].transpose([1, 0, 2]))

            nc.gpsimd.dma_start(out=lwp_s, in_=lwp_d)
            for r in range(N_ROWS):
                gs = row_groups[r]
                c0, c1 = gs[0][4], gs[-1][4] + len(gs[-1][3])
                half = (ROW_TILES[r] + 1) // 2
                dma_band_range(nc.sync, r, 0, half)
                dma_band_range(nc.scalar, r, half, ROW_TILES[r])
                nc.gpsimd.dma_start(
                    out=wt_s[:, c0 * 128:c1 * 128],
                    in_=wt_d[:, c0 * 128:c1 * 128])

            # Linear layer: 4 PSUM column-tile slices per nb; groups are
            # assigned round-robin to col positions (0,32,64,96) and each
            # batch of 4 linear matmuls is emitted back-to-back so they run
            # concurrently in disjoint PE column groups.
            lin_ps = [lin_pool.tile([128, NCHUNK], F32, name=f"lin_ps{nb}")
                      for nb in range(NB)]
            # per (nb, pos): how many groups land there (for start/stop)
            npos = [[0] * 4 for _ in range(NB)]
            for k in range(NG):
                npos[0][k % 4] += 1
                npos[1][k % 4] += 1
            lin_done = [[0] * 4 for _ in range(NB)]

            def emit_lin(gi, nb, stk, k):
                pos = k % 4
                seen = lin_done[nb][pos]
                lin_done[nb][pos] += 1
                nc.tensor.matmul(
                    lin_ps[nb][32 * pos:32 * pos + NCLS, :],
                    lwp_s[:, gi * NCLS:(gi + 1) * NCLS],
                    stk,
                    start=(seen == 0), stop=(seen == npos[nb][pos] - 1),
                    tile_position=(0, 32 * pos),
                    skip_group_check=True,
                )

            relu_i = [0]

            def emit_relu(stk, pp):
                k = relu_i[0] % 2
                relu_i[0] += 1
                if k == 0:
                    nc.vector.tensor_scalar_max(stk, pp, 0.0)
                else:
                    nc.scalar.activation(stk, pp, RELU)

            pend = []
            lin_k = [0, 0]   # per-nb emitted-lin counter (drives col pos)

            def flush_lin(nmin):
                while len(pend) >= nmin:
                    batch = [pend.pop(0) for _ in range(min(4, len(pend)))]
                    for (gi, nb, stk) in batch:
                        emit_lin(gi, nb, stk, lin_k[nb])
                        lin_k[nb] += 1

            for r in range(N_ROWS):
                for nb in range(NB):
                    for (gr, w0, L, ts, ck) in row_groups[r]:
                        gi = GROUPS.index((gr, w0, L, ts, ck))
                        pp = pp_pool.tile([128, NCHUNK], F32, tag="pp")
                        for ci, t in enumerate(ts):
                            bt = r * TPR + t
                            nc.tensor.matmul(
                                pp,
                                wt_s[:, (ck + ci) * 128:(ck + ci + 1) * 128],
                                band_s[:, bt * B + nb * NCHUNK:
                                       bt * B + nb * NCHUNK + NCHUNK],
                                start=(ci == 0), stop=(ci == len(ts) - 1),
                            )
                        stk = stk_pool.tile([128, NCHUNK], BF16, tag="stk")
                        emit_relu(stk, pp)
                        pend.append((gi, nb, stk))
                        flush_lin(6)
            flush_lin(1)
            out_s = sb.tile([106, NB * NCHUNK], F32)
            nc.vector.tensor_copy(out_s[:, 0:NCHUNK], lin_ps[0][0:106, :])
            nc.scalar.activation(out_s[:, NCHUNK:2 * NCHUNK],
                                 lin_ps[1][0:106, :],
                                 mybir.ActivationFunctionType.Copy)
            for nb in range(NB):
                nc.sync.dma_start(
                    out=part_d[nb],
                    in_=out_s[:, nb * NCHUNK:(nb + 1) * NCHUNK])

    nc.compile()
    _cache["nc"] = nc
    return nc


def _core_slots(c):
    """Actual (h, w) per canonical slot for core c; None = pad."""
    slots = []
    for i in range(78):
        slots.append((3 * c + i // 26, i % 26))
    p0 = (52 * c) // 8
    p1 = (52 * (c + 1)) // 8
    ph, pw0 = 24 + p0 // 26, p0 % 26
    plen = p1 - p0
    for j in range(8):
        slots.append((ph, pw0 + j) if j < plen else None)
    return slots, (ph, pw0, plen)


def _prep_core(c, x, W, b, lw4):
    """Build band / wt / lwp arrays for core c."""
    slots, (ph, pw0, plen) = _core_slots(c)

    # bands ------------------------------------------------------------
    hs = [(3 * c, 0), (3 * c + 1, 0), (3 * c + 2, 0), (ph, pw0)]
    band = np.zeros((N_ROWS * TPR, 128, B), dtype=ml_dtypes.bfloat16)
    cj = np.arange(21) // 7          # channel per j
    kij = np.arange(21) % 7          # kernel-row per j
    for r, (h, shift) in enumerate(hs):
        nblocks = min(32, 32 - shift)
        wslice = np.arange(nblocks) + shift
        blk = x[:, cj[:, None], (h + kij)[:, None], wslice[None, :]]
        blk = blk.transpose(1, 2, 0)          # [21, nblocks, B]
        brow = np.zeros((TPR * 128, B), dtype=ml_dtypes.bfloat16)
        for bw in range(nblocks):
            brow[STRIDE * bw:STRIDE * bw + 21] = blk[:, bw]
            brow[STRIDE * bw + 21] = 1.0
        band[r * TPR:(r + 1) * TPR] = brow.reshape(TPR, 128, B)
    # wt ----------------------------------------------------------------
    wt = np.zeros((128, N_CHUNK_TOT * 128), dtype=ml_dtypes.bfloat16)
    for (r, w0, L, ts, ck) in GROUPS:
        for s in range(L):
            w_c = w0 + s
            sl = r * 26 + w_c if r < 3 else 78 + w_c
            hw = slots[sl]
            if hw is None:
                continue
            h, w = hw
            Wl = W[:, :, h, w, :]                 # [42, 3, 49]
            bl = b[:, h, w]                       # [42]
            for ci, t in enumerate(ts):
                col = (ck + ci) * 128 + 42 * s
                rel = 128 * t + np.arange(128) - STRIDE * w_c
                kj = rel // STRIDE
                jj = rel % STRIDE
                valid = (rel >= 0) & (rel < WINLEN) & (jj < 21)
                vals = np.zeros((128, OC), dtype=np.float32)
                vj, vk = jj[valid], kj[valid]
                vals[valid] = Wl[:, vj // 7, (vj % 7) * 7 + vk].T
                bias_row = (rel >= 0) & (rel < WINLEN) & (jj == 21) & (kj == 0)
                if bias_row.any():
                    vals[bias_row] = bl
                wt[:, col:col + OC] = vals.astype(ml_dtypes.bfloat16)
    # lwp ---------------------------------------------------------------
    lwp = np.zeros((128, NG * NCLS), dtype=ml_dtypes.bfloat16)
    for gi, (r, w0, L, ts, ck) in enumerate(GROUPS):
        for s in range(L):
            w_c = w0 + s
            sl = r * 26 + w_c if r < 3 else 78 + w_c
            if slots[sl] is None:
                continue
            h, w = slots[sl]
            lwp[42 * s:42 * s + OC, gi * NCLS:(gi + 1) * NCLS] = (
                lw4[:, :, h, w].T.astype(ml_dtypes.bfloat16)
            )
    return {"band": band, "wt": wt, "lwp": lwp}


def _run(x, W, b, lw, lb, trace=False):
    nc = _build_program()
    x = np.ascontiguousarray(np.asarray(x, dtype=np.float32))
    W = np.asarray(W, dtype=np.float32)
    b = np.asarray(b, dtype=np.float32)
    lw = np.asarray(lw, dtype=np.float32)
    lb = np.asarray(lb, dtype=np.float32)
    lw4 = lw.reshape(NCLS, OC, OH, OW)
    in_maps = [_prep_core(c, x, W, b, lw4) for c in range(NCORES)]
    res = run_bass_kernel_spmd(
        nc, in_maps, list(range(NCORES)), trace=trace,
    )
    part = np.zeros((NB, 106, NCHUNK), dtype=np.float32)
    for c in range(NCORES):
        part += res.results[c]["part"]
    out10 = np.zeros((NCLS, B), dtype=np.float32)
    for nb in range(NB):
        for pos in range(4):
            out10[:, nb * NCHUNK:(nb + 1) * NCHUNK] += (
                part[nb, 32 * pos:32 * pos + NCLS, :])
    out = out10.T + lb[None, :]
    return out.astype(np.float32), res


def kernel(**inputs):
    out, _ = _run(inputs["x"], inputs["W"], inputs["b"], inputs["lw"],
                  inputs["lb"])
    return out


# revision 14
# speedup vs baseline: 1.6156x; 1.1394x over previous
"""Trainium2 Bass kernel for nn_Net2_54494545051831 (LocallyConnected2d(7x7)
-> bias -> ReLU -> Linear(28392 -> 10)), on 8 NeuronCores.

Distribution: by output location. Each core owns 3 full output rows
(h = 3c .. 3c+2) plus a 6-7 wide piece of rows 24/25 -> 84/85 locations.
Weights / bias / lw are sharded by location (nothing replicated); each core
computes a partial [10, B] of the final linear layer, summed on host.

Per-core compute ("band" layout): for each owned output row, x is reordered
host-side so the contraction rows of location (h, w) sit at band partitions
[22w, 22w+153): band row 22*w' + j = x[:, j//7, h + j%7, w'] for j < 21,
1.0 at j == 21 (bias folds into the weights), 0 above.

Locations are processed in GROUPS of 3 (42*3 = 126 output channels + 2 zero
columns = full 128-wide stationary operand). A group's 3 windows span
[22w0, 22w0+197) -> 2-3 aligned 128-row band tiles; one full-width matmul
per (group, tile) with zero weights on rows outside each location's window.
ReLU round-robins across Vector/Scalar/GpSimd; the linear layer contracts
each group's relu'd [128, 512] tile with a [128, 10] per-group lw slice,
accumulating in PSUM. bf16 matmuls with fp32 accumulation.
"""
import numpy as np
import ml_dtypes

import concourse.mybir as mybir
import concourse.tile as tile
from concourse import bacc
from concourse.bass_utils import run_bass_kernel_spmd

BF16 = mybir.dt.bfloat16
F32 = mybir.dt.float32
RELU = mybir.ActivationFunctionType.Relu

B = 1024
IC, OC, NCLS = 3, 42, 10
KH = KW = 7
OH = OW = 26
NCORES = 8
N_ROWS = 4           # canonical band rows per core (3 full + 1 piece)
STRIDE = 22          # band rows per w'-block: 21 data + 1 ones(bias) row
WINLEN = 6 * STRIDE + 21   # partition span of one location window (153)
TPR = 6              # band tiles per canonical row (704 rows -> 6 tiles)
NB = 2               # two N-chunks of 512
NCHUNK = 512
N_WARM = 16          # PE warm-up matmuls during the DMA prologue

# Groups of consecutive locations within a canonical row: (w0, len)
GROUPS_FULL = [(0, 3), (3, 3), (6, 3), (9, 3), (12, 3), (15, 3), (18, 3),
               (21, 3), (24, 2)]
GROUPS_ROW3 = [(0, 3), (3, 3), (6, 2)]


def _group_tiles(w0, L):
    ta = (STRIDE * w0) // 128
    tb = (STRIDE * (w0 + L - 1) + WINLEN - 1) // 128
    return list(range(ta, tb + 1))


def _groups():
    """[(row, w0, L, [tiles], chunk0)] — chunk0 = first wt chunk index."""
    out = []
    ck = 0
    for r in range(N_ROWS):
        for w0, L in (GROUPS_FULL if r < 3 else GROUPS_ROW3):
            ts = _group_tiles(w0, L)
            out.append((r, w0, L, ts, ck))
            ck += len(ts)
    return out, ck

GROUPS, N_CHUNK_TOT = _groups()
NG = len(GROUPS)
# band tiles actually used per canonical row (row 3 only needs tiles 0-2)
ROW_TILES = [TPR, TPR, TPR, max(t for (r, _, _, ts, _) in GROUPS if r == 3
                                for t in ts) + 1]

_cache = {}


def _build_program():
    if "nc" in _cache:
        return _cache["nc"]

    nc = bacc.Bacc("TRN2", target_bir_lowering=False, debug=False,
                   num_devices=NCORES)
    band_d = nc.dram_tensor("band", [N_ROWS * TPR, 128, B], BF16,
                            kind="ExternalInput").ap()
    wt_d = nc.dram_tensor("wt", [128, N_CHUNK_TOT * 128], BF16,
                          kind="ExternalInput").ap()
    lwp_d = nc.dram_tensor("lwp", [128, NG * NCLS], BF16,
                           kind="ExternalInput").ap()
    # 4 col-tile partial slices per nb chunk, summed on host
    part_d = nc.dram_tensor("part", [NB, 4, NCLS, NCHUNK], F32,
                            kind="ExternalOutput").ap()

    with tile.TileContext(nc) as tc:
        with (
            tc.tile_pool(name="sb", bufs=1) as sb,
            tc.tile_pool(name="stk_pool", bufs=8) as stk_pool,
            tc.tile_pool(name="pp_pool", bufs=5, space="PSUM") as pp_pool,
            tc.tile_pool(name="lin_pool", bufs=1, space="PSUM") as lin_pool,
            tc.tile_pool(name="warm_pool", bufs=1, space="PSUM") as warm_pool,
        ):
            band_s = sb.tile([128, N_ROWS * TPR * B], BF16)
            wt_s = sb.tile([128, N_CHUNK_TOT * 128], BF16)
            lwp_s = sb.tile([128, NG * NCLS], BF16)
            zz = sb.tile([128, NCHUNK], BF16)

            # PE warm-up: zero tile matmuls with no DMA deps — scheduled
            # during the DMA prologue so HAM reaches K=8/8 before real work.
            nc.vector.memset(zz, 0.0)
            warm_ps = warm_pool.tile([128, NCHUNK], F32, name="warm_ps")
            for _ in range(N_WARM):
                nc.tensor.matmul(warm_ps, zz[:, 0:128], zz,
                                 start=True, stop=True)

            # DMA: few large transfers, software-pipelined. Row 0 upfront;
            # row r+1's triggers are emitted mid-loop behind row r's first
            # relu (scalar/gpsimd FIFO order gates them), so in-flight
            # transfers never steal bandwidth from the row the PE needs now.
            row_groups = [[g for g in GROUPS if g[0] == r]
                          for r in range(N_ROWS)]

            def dma_band_range(r, t0, t1):
                if t0 >= t1:
                    return
                b0 = r * TPR + t0
                nc.scalar.dma_start(
                    out=band_s[:, b0 * B:(r * TPR + t1) * B],
                    in_=band_d[b0:r * TPR + t1].transpose([1, 0, 2]))

            def dma_row(r):
                gs = row_groups[r]
                c0, c1 = gs[0][4], gs[-1][4] + len(gs[-1][3])
                half = (ROW_TILES[r] + 1) // 2
                dma_band_range(r, 0, half)
                dma_band_range(r, half, ROW_TILES[r])
                nc.gpsimd.dma_start(
                    out=wt_s[:, c0 * 128:c1 * 128],
                    in_=wt_d[:, c0 * 128:c1 * 128])

            nc.gpsimd.dma_start(out=lwp_s, in_=lwp_d)
            dma_row(0)

            # Linear layer: 4 PSUM column-tile slices per nb; groups are
            # assigned round-robin to col positions (0,32,64,96) and each
            # batch of 4 linear matmuls is emitted back-to-back so they run
            # concurrently in disjoint PE column groups.
            lin_ps = [lin_pool.tile([128, NCHUNK], F32, name=f"lin_ps{nb}")
                      for nb in range(NB)]
            # per (nb, pos): how many groups land there (for start/stop)
            npos = [[0] * 4 for _ in range(NB)]
            for k in range(NG):
                npos[0][k % 4] += 1
                npos[1][k % 4] += 1
            lin_done = [[0] * 4 for _ in range(NB)]

            def emit_lin(gi, nb, stk, k):
                pos = k % 4
                seen = lin_done[nb][pos]
                lin_done[nb][pos] += 1
                nc.tensor.matmul(
                    lin_ps[nb][32 * pos:32 * pos + NCLS, :],
                    lwp_s[:, gi * NCLS:(gi + 1) * NCLS],
                    stk,
                    start=(seen == 0), stop=(seen == npos[nb][pos] - 1),
                    tile_position=(0, 32 * pos),
                    skip_group_check=True,
                )

            relu_i = [0]

            def emit_relu(stk, pp):
                k = relu_i[0] % 2
                relu_i[0] += 1
                if k == 0:
                    nc.vector.tensor_scalar_max(stk, pp, 0.0)
                else:
                    nc.scalar.activation(stk, pp, RELU)

            pend = []
            lin_k = [0, 0]   # per-nb emitted-lin counter (drives col pos)

            def flush_lin(nmin):
                while len(pend) >= nmin:
                    batch = [pend.pop(0) for _ in range(min(4, len(pend)))]
                    for (gi, nb, stk) in batch:
                        emit_lin(gi, nb, stk, lin_k[nb])
                        lin_k[nb] += 1

            for r in range(N_ROWS):
                for nb in range(NB):
                    for gidx, (gr, w0, L, ts, ck) in enumerate(row_groups[r]):
                        gi = GROUPS.index((gr, w0, L, ts, ck))
                        pp = pp_pool.tile([128, NCHUNK], F32, tag="pp")
                        for ci, t in enumerate(ts):
                            bt = r * TPR + t
                            nc.tensor.matmul(
                                pp,
                                wt_s[:, (ck + ci) * 128:(ck + ci + 1) * 128],
                                band_s[:, bt * B + nb * NCHUNK:
                                       bt * B + nb * NCHUNK + NCHUNK],
                                start=(ci == 0), stop=(ci == len(ts) - 1),
                            )
                        stk = stk_pool.tile([128, NCHUNK], BF16, tag="stk")
                        emit_relu(stk, pp)
                        if nb == 0 and gidx == 0 and r + 1 < N_ROWS:
                            dma_row(r + 1)
                        pend.append((gi, nb, stk))
                        flush_lin(6)
            flush_lin(1)
            out_s = sb.tile([106, NB * NCHUNK], F32)
            nc.vector.tensor_copy(out_s[:, 0:NCHUNK], lin_ps[0][0:106, :])
            nc.scalar.activation(out_s[:, NCHUNK:2 * NCHUNK],
                                 lin_ps[1][0:106, :],
                                 mybir.ActivationFunctionType.Copy)
            oeng = [nc.sync, nc.gpsimd, nc.scalar]
            for nb in range(NB):
                for pos in range(4):
                    oeng[(nb * 4 + pos) % 3].dma_start(
                        out=part_d[nb][pos],
                        in_=out_s[32 * pos:32 * pos + NCLS,
                                  nb * NCHUNK:(nb + 1) * NCHUNK])

    nc.compile()
    _cache["nc"] = nc
    return nc


def _core_slots(c):
    """Actual (h, w) per canonical slot for core c; None = pad."""
    slots = []
    for i in range(78):
        slots.append((3 * c + i // 26, i % 26))
    p0 = (52 * c) // 8
    p1 = (52 * (c + 1)) // 8
    ph, pw0 = 24 + p0 // 26, p0 % 26
    plen = p1 - p0
    for j in range(8):
        slots.append((ph, pw0 + j) if j < plen else None)
    return slots, (ph, pw0, plen)


def _prep_core(c, x, W, b, lw4):
    """Build band / wt / lwp arrays for core c."""
    slots, (ph, pw0, plen) = _core_slots(c)

    # bands ------------------------------------------------------------
    hs = [(3 * c, 0), (3 * c + 1, 0), (3 * c + 2, 0), (ph, pw0)]
    band = np.zeros((N_ROWS * TPR, 128, B), dtype=ml_dtypes.bfloat16)
    cj = np.arange(21) // 7          # channel per j
    kij = np.arange(21) % 7          # kernel-row per j
    for r, (h, shift) in enumerate(hs):
        nblocks = min(32, 32 - shift)
        wslice = np.arange(nblocks) + shift
        blk = x[:, cj[:, None], (h + kij)[:, None], wslice[None, :]]
        blk = blk.transpose(1, 2, 0)          # [21, nblocks, B]
        brow = np.zeros((TPR * 128, B), dtype=ml_dtypes.bfloat16)
        for bw in range(nblocks):
            brow[STRIDE * bw:STRIDE * bw + 21] = blk[:, bw]
            brow[STRIDE * bw + 21] = 1.0
        band[r * TPR:(r + 1) * TPR] = brow.reshape(TPR, 128, B)
    # wt ----------------------------------------------------------------
    wt = np.zeros((128, N_CHUNK_TOT * 128), dtype=ml_dtypes.bfloat16)
    for (r, w0, L, ts, ck) in GROUPS:
        for s in range(L):
            w_c = w0 + s
            sl = r * 26 + w_c if r < 3 else 78 + w_c
            hw = slots[sl]
            if hw is None:
                continue
            h, w = hw
            Wl = W[:, :, h, w, :]                 # [42, 3, 49]
            bl = b[:, h, w]                       # [42]
            for ci, t in enumerate(ts):
                col = (ck + ci) * 128 + 42 * s
                rel = 128 * t + np.arange(128) - STRIDE * w_c
                kj = rel // STRIDE
                jj = rel % STRIDE
                valid = (rel >= 0) & (rel < WINLEN) & (jj < 21)
                vals = np.zeros((128, OC), dtype=np.float32)
                vj, vk = jj[valid], kj[valid]
                vals[valid] = Wl[:, vj // 7, (vj % 7) * 7 + vk].T
                bias_row = (rel >= 0) & (rel < WINLEN) & (jj == 21) & (kj == 0)
                if bias_row.any():
                    vals[bias_row] = bl
                wt[:, col:col + OC] = vals.astype(ml_dtypes.bfloat16)
    # lwp ---------------------------------------------------------------
    lwp = np.zeros((128, NG * NCLS), dtype=ml_dtypes.bfloat16)
    for gi, (r, w0, L, ts, ck) in enumerate(GROUPS):
        for s in range(L):
            w_c = w0 + s
            sl = r * 26 + w_c if r < 3 else 78 + w_c
            if slots[sl] is None:
                continue
            h, w = slots[sl]
            lwp[42 * s:42 * s + OC, gi * NCLS:(gi + 1) * NCLS] = (
                lw4[:, :, h, w].T.astype(ml_dtypes.bfloat16)
            )
    return {"band": band, "wt": wt, "lwp": lwp}


def _run(x, W, b, lw, lb, trace=False):
    nc = _build_program()
    x = np.ascontiguousarray(np.asarray(x, dtype=np.float32))
    W = np.asarray(W, dtype=np.float32)
    b = np.asarray(b, dtype=np.float32)
    lw = np.asarray(lw, dtype=np.float32)
    lb = np.asarray(lb, dtype=np.float32)
    lw4 = lw.reshape(NCLS, OC, OH, OW)
    in_maps = [_prep_core(c, x, W, b, lw4) for c in range(NCORES)]
    res = run_bass_kernel_spmd(
        nc, in_maps, list(range(NCORES)), trace=trace,
    )
    part = np.zeros((NB, 4, NCLS, NCHUNK), dtype=np.float32)
    for c in range(NCORES):
        part += res.results[c]["part"]
    out10 = part.sum(axis=1).transpose(1, 0, 2).reshape(NCLS, B)
    out = out10.T + lb[None, :]
    return out.astype(np.float32), res


def kernel(**inputs):
    out, _ = _run(inputs["x"], inputs["W"], inputs["b"], inputs["lw"],
                  inputs["lb"])
    return out
